# revision 49
# baseline (speedup 1.0000x reference)
"""Multi-head attention (B=2, N=4096, C=512, H=8) on 8 TRN2 NeuronCores.

Sharding: core c handles batch c//4 and heads {2*(c%4), 2*(c%4)+1}
(data parallel over batch, tensor parallel over heads). Each core
computes its 2 heads' attention plus a partial output projection;
the host sums the 4 partials per batch and adds the bias terms
(b_out and b_v @ W_out, which commutes past softmax-weighted sums).

Per-core compute (matmul operands bf16, all accumulation f32):
  xT   host-pretransposed, streamed in per 512-token block
  kT   both heads stacked on K=128 (one M=128 projection per block)
  qT   both heads stacked on K=128 (one M=128 projection per block);
       q weights/biases host-scaled by 1/sqrt(C) so scores arrive
       pre-scaled and exp runs with scale=1
  v    projected straight into PV-operand layout (lhsT = xT block),
       with a memset ones column so P.T@[v|1] yields softmax sums free
  attention, one flat software-pipelined stream over (qb, kb):
     scoresT for BOTH heads concurrently via PE row tiling: two K=64
     matmuls at tile_position (0,0) and (64,0) sharing the stacked qT
     stream -> PSUM ks [128, 2, 512] f32 (one bank per head); the two
     MMs start ~6ns apart and run on disjoint row groups, so a head's
     scores cost half of the zero-padded K=128 formulation
     expT = Exp(scoresT) -> SBUF bf16 (one wide ACT op, FD=1024); the
     scalar engine is the bottleneck of the whole kernel (~262us of
     exp at 1 elem/cycle/lane; nothing else can compute exp -- custom
     DVE ops are unsupported by this walrus), so everything else is
     arranged to keep its stream gapless
     accum v_aug.T @ expT per head over kb -> PSUM po_h [65, 512];
     PV for (kb, h) lags the scores by 3+h flat steps and the queue
     DRAINS ACROSS qb boundaries, so the PE chews leftover PVs while
     the first exp of the new qb frees the scores ring
  deferred tails (per qb, running during the next qb so the in-order
  engine FIFOs never head-block the hot stream):
     po evac copy fires INLINE right after that head's last PV (frees
     the bank before the next qb's PV rotates onto it); g4: [1,512]
     iterative reciprocal of the sums row (DVE); g12: DRAM round trip
     + SWDGE broadcast-read across 64 partitions, normalize (the DMAs
     ride the otherwise-idle GPSIMD queue); g15/17/19/21: output
     projection (both heads accumulated, K=128 zero-padded) + store
     For the LAST qb the two reciprocals would sit on the exit
     critical path, so both sums rows round-trip through DRAM into a
     [128, 8] relayout for one ~0.2us reciprocal, the chain rides the
     by-then-idle sync queue, and ~35 dummy matmuls keep the HAM
     clock gate at 2.4 GHz through the round trip (else the
     projections run cold at 1.2 GHz)
  qkv production for block t is interleaved at groups 4(t-1)/4(t-1)+2
  of qb 0 (k+v01 then v23+q), sharing the projection PSUM tag.

PSUM: ks 2 bufs x 2 banks + po/warmt 2 bufs x 1 bank + production/
proj "pp" 2 bufs x 1 bank = 8 banks exactly.
"""

import numpy as np

import concourse.bass as bass
import concourse.mybir as mybir
import concourse.tile as tile
from concourse.bass_utils import run_bass_kernel_spmd
from concourse.tile_rust import add_dep_helper
from concourse.vector_clock import ScopedClock

F32 = mybir.dt.float32
BF16 = mybir.dt.bfloat16
AF = mybir.ActivationFunctionType

B, N, C, H = 2, 4096, 512, 8
HD = C // H          # 64
HPC = H // 4         # 2 heads per core
NCORES = 8
NT = N // 128        # 32 key chunks
NCJ = C // 128       # 4 contraction chunks
QB = N // 512        # 8 query blocks
SCALE = 1.0 / float(np.sqrt(C))


def _patch_tail_drain():
    """This walrus build caps sync waits at 1 per non-EventSemaphore
    instruction (2 for EventSemaphore); the stock TileContext tail-drain
    attaches every outstanding wait to one Drain, and the scheduler can
    leave >1 wait on regular instructions. Spill extras onto fresh
    same-engine nops inserted just before the over-subscribed one."""
    if getattr(tile.TileContext, "_drain_patched", False):
        return

    def _spill_excess_waits(nc):
        for fn in nc.m.functions:
            for bb in fn.blocks:
                insts = bb.instructions
                i = 0
                while i < len(insts):
                    inst = insts[i]
                    si = inst.sync_info
                    cap = 2 if isinstance(inst, mybir.InstEventSemaphore) else 1
                    if si is None or len(si.on_wait) <= cap:
                        i += 1
                        continue
                    extra = list(si.on_wait[cap:])
                    si.on_wait[:] = si.on_wait[:cap]
                    for w in extra:
                        nop = nc.engines[inst.engine].nop(
                            hint="wait_spill", nofuse=True
                        )
                        cur = nc.cur_bb.bb.instructions
                        cur.remove(nop.ins)
                        if nop.ins.sync_info is None:
                            nop.ins.sync_info = mybir.SyncInfo(
                                on_update=[], on_wait=[]
                            )
                        nop.ins.sync_info.on_wait.append(w)
                        insts.insert(i, nop.ins)
                        i += 1
                    i += 1

    def _drain_and_barrier(self, tick_clock, wait_clock):
        nc = self.nc
        drain_inst = nc.sync.drain()
        wait_clock.add_sem_waits(
            drain_inst.ins, ScopedClock({None: tick_clock.global_clock})
        )
        nc.all_engine_barrier()
        assert self.sems is not None
        popped = nc._tile_sem_poison_stack.pop()
        assert popped is self._sem_poison
        nc.clear_and_free_semaphores(list(self.sems.allocated().values()))
        # no second all-engine barrier: after the quiesce + sem clears
        # nothing needs cross-engine ordering before program end; the
        # final rendezvous costs ~3us of pure teardown
        _spill_excess_waits(nc)

    tile.TileContext._drain_and_barrier = _drain_and_barrier
    tile.TileContext._drain_patched = True


def _build_program():
    _patch_tail_drain()
    nc = bass.Bass()

    xt = nc.dram_tensor("xt", [QB, 128, NCJ, 512], BF16, kind="ExternalInput")
    # host-prearranged weight layouts (see kernel() below); both heads are
    # packed into one M=128 projection for q (rows 0:64 = h0, 64:128 = h1),
    # matching the stacked kT rows so the row-tiled scores matmuls read
    # aligned 64-partition slices of one qT tile
    w_q = nc.dram_tensor("w_q", [128, NCJ, 128], BF16, kind="ExternalInput")
    w_k = nc.dram_tensor("w_k", [128, NCJ, 128], BF16, kind="ExternalInput")
    w_v = nc.dram_tensor("w_v", [128, NCJ, HPC * HD], BF16, kind="ExternalInput")
    w_o = nc.dram_tensor("w_o", [128, HPC, C], BF16, kind="ExternalInput")
    b_q = nc.dram_tensor("b_q", [128, 1], F32, kind="ExternalInput")
    b_k = nc.dram_tensor("b_k", [128, 1], F32, kind="ExternalInput")
    out = nc.dram_tensor("out", [N, C], F32, kind="ExternalOutput")
    scratch = nc.dram_tensor("scratch", [QB * HPC + 2, 512], F32)

    from contextlib import ExitStack

    with tile.TileContext(nc) as tc, ExitStack() as ctx:
        # PE warmup: ~4us of back-to-back dummy matmuls during the initial
        # DMA wait so the HAM clock gate reaches K=8/8 (2.4 GHz) before the
        # first real matmul; garbage operand values are fine.
        const = ctx.enter_context(tc.tile_pool(name="const", bufs=1))
        w_q_sb = const.tile([128, NCJ, 128], BF16)
        w_k_sb = const.tile([128, NCJ, 128], BF16)
        w_v_sb = const.tile([128, NCJ, HPC * HD], BF16)
        w_o_sb = const.tile([128, HPC, C], BF16)
        b_q_sb = const.tile([128, 1], F32)
        b_k_sb = const.tile([128, 1], F32)

        persist = ctx.enter_context(tc.tile_pool(name="persist", bufs=1))
        # qT/kT hold BOTH heads stacked (rows 0:64 = h0, 64:128 = h1); the
        # scores row-tiling selects the head via the K=64 partition slice
        qT = persist.tile([128, N], BF16)
        kT = persist.tile([128, N], BF16)
        # [tokens, kb, head, 128]: dims at 0:64, ones at 64 (from memset)
        v_nat = persist.tile([128, NT, HPC, 128], BF16)

        # ---- fused pipeline: qkv production interleaved into attention ----
        with (
            tc.tile_pool(name="xTp", bufs=1) as xTp,
            tc.tile_pool(name="oTp", bufs=1) as oTp,
            tc.tile_pool(name="expp", bufs=6) as expp,
            tc.tile_pool(name="posb", bufs=3) as posb,
            tc.tile_pool(name="recipp", bufs=3) as recipp,
            tc.tile_pool(name="bcsb", bufs=3) as bcsb,
            tc.tile_pool(name="ostage", bufs=4) as ostage,
            tc.tile_pool(name="ps_s", bufs=2, space="PSUM") as ps_s,
            tc.tile_pool(name="ps_o", bufs=2, space="PSUM") as ps_o,
            tc.tile_pool(name="ps_p", bufs=2, space="PSUM") as ps_p,
        ):
            # oT zero-padded to K=128 (rows 64:128 stay 0; w_o rows there are
            # host-zeroed) so the projection avoids the K=64/M=128 slow path
            oT = oTp.tile([128, HPC, N], BF16)
            xT = xTp.tile([128, NCJ, N], BF16)
            # DMA waits are precise per-transfer semaphores; what gates the
            # head is TRANSFER order. Small weights go first on the sync
            # queue while xT block 0 streams per-cj IN PARALLEL on the
            # gpsimd queue (subtile deps let the first k-matmul start
            # after w_k + just the cj0 slice). The big oT memset is
            # emitted after the xT loads so it doesn't block the gpsimd
            # descriptor stream (it's only needed by ~55us in).
            nc.sync.dma_start(out=w_k_sb, in_=w_k[:])
            nc.sync.dma_start(out=b_k_sb, in_=b_k[:])
            nc.sync.dma_start(out=w_q_sb, in_=w_q[:])
            nc.sync.dma_start(out=b_q_sb, in_=b_q[:])
            nc.sync.dma_start(out=w_v_sb, in_=w_v[:])
            nc.sync.dma_start(out=w_o_sb, in_=w_o[:])
            for cj in range(NCJ):
                nc.gpsimd.dma_start(
                    out=xT[:, cj, 0:512], in_=xt[0][:, cj, :]
                )
            for tb in range(1, QB):
                tsl = slice(tb * 512, (tb + 1) * 512)
                nc.gpsimd.dma_start(out=xT[:, :, tsl], in_=xt[tb])
            nc.gpsimd.memset(oT[HD:128, :, :], 0.0)
            # only the ones column needs initializing: PV reads cols 0:65
            nc.vector.memset(v_nat[:, :, :, HD:HD + 1], 1.0)

            last_pe = {"inst": None}

            def pe_keepwarm(n):
                """Dummy matmuls to hold the HAM clock gate at K=8/8.
                Reading w_k_sb pins them after its DMA (so the PE warms
                during the load, right before the first real matmul);
                chaining off last_pe pins tail bursts after the last PV."""
                wt = ps_o.tile([128, 512], F32, name="warmt", tag="po")
                for _ in range(n):
                    mm = nc.tensor.matmul(
                        wt, lhsT=w_k_sb[:, 0, :], rhs=qT[:, 0:512],
                        start=True, stop=True,
                    )
                    if last_pe["inst"] is not None:
                        add_dep_helper(
                            mm.ins, last_pe["inst"], sync=False,
                            reason="keepwarm pinned to tail",
                        )
                    last_pe["inst"] = mm.ins

            def vnat_kb(kb):
                ksl = slice(kb * 128, (kb + 1) * 128)
                pv_ = ps_p.tile([128, HPC * HD], F32, tag="pp", name="pv_")
                for cj in range(NCJ):
                    nc.tensor.matmul(
                        pv_,
                        lhsT=xT[:, cj, ksl],
                        rhs=w_v_sb[:, cj, :],
                        start=(cj == 0),
                        stop=(cj == NCJ - 1),
                    )
                nc.vector.tensor_copy(
                    out=v_nat[:, kb, :, 0:HD],
                    in_=pv_.rearrange("p (h d) -> p h d", h=HPC),
                )

            def production(tb, part=None):
                """qkv projections for one 512-token block; psum via the
                shared 'pp' tag (temporally disjoint from proj use).
                parts: 0=k, 1=v01, 2=v23, 3=q (one per group, spread so
                qb0's per-group PE burden stays under the ACT pace);
                "head": k+q then all v (block-0 path)."""
                tsl = slice(tb * 512, (tb + 1) * 512)
                if part in (0, None, "head"):
                    pk = ps_p.tile([128, 512], F32, tag="pp", name="pk")
                    for cj in range(NCJ):
                        nc.tensor.matmul(
                            pk,
                            lhsT=w_k_sb[:, cj, :],
                            rhs=xT[:, cj, tsl],
                            start=(cj == 0),
                            stop=(cj == NCJ - 1),
                        )
                    nc.vector.tensor_scalar_add(
                        out=kT[:, tsl], in0=pk, scalar1=b_k_sb
                    )
                if part in (1, None):
                    for kb in range(tb * 4, tb * 4 + 2):
                        vnat_kb(kb)
                if part in (2, None):
                    for kb in range(tb * 4 + 2, tb * 4 + 4):
                        vnat_kb(kb)
                if part in (3, None, "head"):
                    pm = ps_p.tile([128, 512], F32, tag="pp", name="pm")
                    for cj in range(NCJ):
                        nc.tensor.matmul(
                            pm,
                            lhsT=w_q_sb[:, cj, :],
                            rhs=xT[:, cj, tsl],
                            start=(cj == 0),
                            stop=(cj == NCJ - 1),
                        )
                    nc.vector.tensor_scalar_add(
                        out=qT[:, tsl], in0=pm, scalar1=b_q_sb
                    )
                if part == "head":
                    for kb in range(tb * 4, tb * 4 + 4):
                        vnat_kb(kb)

            # block 0: k and q first (the first scores pair needs only
            # those), v afterwards -- the first exp starts ~3us sooner
            production(0, part="head")
            last_exp = {"inst": None}
            last_pe = {"inst": None}
            evac_by = {}        # (qb, h) -> copy thunk, fired inline right
                                # after that head's LAST PV (so the bank is
                                # freed before the next qb rotates onto it,
                                # and never copied before the accum is done)
            pending_recip = []  # flushed @g4 of the following qb (DVE)
            pending_bc = []     # flushed @g12 (SWDGE bcast + DVE normalize)
            pending_proj = []   # flushed @g15/17/19/21 (PE matmuls)

            fin = {"wr0": [], "wr1": None}

            def make_tail(po, h, qsl, u2, final=False):
                """Tail queue discipline: the evac COPY frees the PSUM bank
                (emitted for both heads before either fat reciprocal so the
                next qb's PV never waits behind a 3.4us recip in the DVE
                FIFO); the scratch write rides the GPSIMD queue (its only
                other traffic is this tail) so the cross-engine wait never
                head-blocks the out-store DMA queue."""
                state = {}

                def evac_copy():
                    ps = posb.tile([HD + 1, 512], F32, name="ps")
                    nc.vector.tensor_copy(out=ps, in_=po)
                    state["ps"] = ps

                def evac_recip():
                    if final:
                        # last qb: the 3.4us/head one-lane reciprocal sits
                        # on the exit critical path -- spread BOTH heads'
                        # sums across 128 partitions via a DRAM relayout
                        # and do one [128, 8] reciprocal (~0.2us)
                        fin["wr0"].append(nc.sync.dma_start(
                            out=scratch[u2:u2 + 1, :],
                            in_=state["ps"][HD:HD + 1, :],
                        ))
                        if len(fin["wr0"]) == 2:
                            base = (QB * HPC - HPC) * 512
                            rs = recipp.tile(
                                [128, 8], F32, name="rs", tag="rs"
                            )
                            rd0 = nc.sync.dma_start(
                                out=rs,
                                in_=bass.AP(
                                    tensor=scratch, offset=base,
                                    ap=[[8, 128], [1, 8]],
                                ),
                            )
                            for w in fin["wr0"]:
                                add_dep_helper(
                                    rd0.ins, w.ins, sync=True,
                                    reason="final sums relayout RAW",
                                )
                            rr = recipp.tile(
                                [128, 8], F32, name="rr", tag="rr"
                            )
                            nc.vector.reciprocal(out=rr, in_=rs)
                            fin["wr1"] = nc.sync.dma_start(
                                out=bass.AP(
                                    tensor=scratch,
                                    offset=(QB * HPC) * 512,
                                    ap=[[8, 128], [1, 8]],
                                ),
                                in_=rr,
                            )
                        return
                    rt = recipp.tile([1, 512], F32, name="rt")
                    nc.vector.reciprocal(
                        out=rt, in_=state["ps"][HD:HD + 1, :]
                    )
                    # round-trip through DRAM to broadcast across
                    # partitions (no on-chip partition-broadcast path)
                    state["wr"] = nc.gpsimd.dma_start(
                        out=scratch[u2:u2 + 1, :], in_=rt
                    )

                def bcmult(block=None):
                    bc = state.get("bc")
                    if bc is None:
                        src_row = (QB * HPC + h) if final else u2
                        bc = bcsb.tile([HD, 512], F32, name="bc")
                        dma_eng = nc.sync if final else nc.gpsimd
                        rd = dma_eng.dma_start(
                            out=bc,
                            in_=bass.AP(
                                tensor=scratch, offset=src_row * 512,
                                ap=[[0, HD], [1, 512]],
                            ),
                        )
                        add_dep_helper(
                            rd.ins,
                            (fin["wr1"] if final else state["wr"]).ins,
                            sync=True,
                            reason="recip broadcast RAW",
                        )
                        state["bc"] = bc
                    csl = slice(0, 512) if block is None else slice(
                        block * 128, (block + 1) * 128
                    )
                    osl = slice(qsl.start + csl.start, qsl.start + csl.stop)
                    nc.vector.tensor_mul(
                        out=oT[0:HD, h, osl],
                        in0=state["ps"][0:HD, csl], in1=bc[:, csl],
                    )
                return evac_copy, evac_recip, bcmult

            def make_proj(qb, j):
                def proj():
                    q0 = qb * 512 + j * 128
                    pp = ps_p.tile([128, C], F32, tag="pp", name="pp")
                    for h in range(HPC):
                        mm = nc.tensor.matmul(
                            pp,
                            lhsT=oT[:, h, q0:q0 + 128],
                            rhs=w_o_sb[:, h, :],
                            start=(h == 0),
                            stop=(h == HPC - 1),
                        )
                        if h == 0 and last_exp["inst"] is not None:
                            add_dep_helper(
                                mm.ins, last_exp["inst"], sync=False,
                                reason="proj after normalize really done",
                            )
                    ot = ostage.tile([128, C], F32, name="ot")
                    nc.vector.tensor_copy(out=ot, in_=pp)
                    nc.sync.dma_start(out=out[q0:q0 + 128, :], in_=ot)
                return proj

            def s_pair(qb, kb):
                """Both heads' scoresT for one key chunk, concurrently on
                the PE via row tiling (K=64 each, shared qT stream)."""
                qsl = slice(qb * 512, (qb + 1) * 512)
                ksl = slice(kb * 128, (kb + 1) * 128)
                ks = ps_s.tile([128, HPC, 512], F32, name="ks")
                for h in range(HPC):
                    nc.tensor.matmul(
                        ks[:, h, :],
                        lhsT=kT[h * HD:(h + 1) * HD, ksl],
                        rhs=qT[h * HD:(h + 1) * HD, qsl],
                        start=True,
                        stop=True,
                        tile_position=(h * HD, 0),
                    )
                return ks

            # flat (qb, kb) pipeline: the scores skew AND the lagged PV
            # queue carry across qb boundaries so the PE/ACT streams never
            # drain (the PE chews leftover PVs while the first exp of the
            # new qb frees the scores ring)
            flat = [(qb, kb) for qb in range(QB) for kb in range(NT)]
            po_tiles = {}
            pending_pv = []   # (i, qb, kb, h, et); lag 3+h flat steps

            def pv_one(qb_, kb_, h_, et_):
                if (qb_, h_) not in po_tiles:
                    po_tiles[(qb_, h_)] = ps_o.tile(
                        [HD + 1, 512], F32, name="po"
                    )
                mm = nc.tensor.matmul(
                    po_tiles[(qb_, h_)],
                    lhsT=v_nat[:, kb_, h_, 0:HD + 1],
                    rhs=et_[:, h_, :],
                    start=(kb_ == 0),
                    stop=(kb_ == NT - 1),
                )
                last_pe["inst"] = mm.ins
                if kb_ == NT - 1 and (qb_, h_) in evac_by:
                    evc, rec = evac_by.pop((qb_, h_))
                    evc()
                    pending_recip.append(rec)

            pend = s_pair(*flat[0])
            for i, (qb, kb) in enumerate(flat):
                ks = pend
                pend = s_pair(*flat[i + 1]) if i + 1 < len(flat) else None
                g = kb
                if g == 4:
                    for r in pending_recip:
                        r()
                    pending_recip.clear()
                if qb == 0 and g % 2 == 0 and g // 4 + 1 < QB:
                    production(g // 4 + 1, part=g % 4)
                    production(g // 4 + 1, part=g % 4 + 1)
                if g == 12:
                    for f in pending_bc:
                        f()
                    pending_bc.clear()
                elif g in (15, 17, 19, 21) and pending_proj:
                    pending_proj.pop(0)()
                et = expp.tile([128, HPC, 512], BF16)
                exp_bi = nc.scalar.activation(
                    out=et, in_=ks, func=AF.Exp, scale=1.0
                )
                last_exp["inst"] = exp_bi.ins

                for h in range(HPC):
                    pending_pv.append((i, qb, kb, h, et))
                while pending_pv and i - pending_pv[0][0] >= 3 + pending_pv[0][3]:
                    pv_one(*pending_pv.pop(0)[1:])
                if kb == NT - 1:
                    qsl = slice(qb * 512, (qb + 1) * 512)
                    for h in range(HPC):
                        evc, rec, bcm = make_tail(
                            po_tiles[(qb, h)], h, qsl, qb * HPC + h,
                            final=(qb == QB - 1),
                        )
                        evac_by[(qb, h)] = (evc, rec)
                        pending_bc.append(bcm)
                    for j in range(4):
                        pending_proj.append(make_proj(qb, j))
            while pending_pv:
                pv_one(*pending_pv.pop(0)[1:])
            # --- final-qb tail: keep the PE warm through the reciprocal
            # round trip (else HAM re-throttles and the projections run at
            # 1.2 GHz), and interleave per-128-block normalize with the
            # projections to shorten the serial chain ---
            for r in pending_recip:
                r()
            pending_recip.clear()
            pe_keepwarm(46)
            for j in range(4):
                for f in pending_bc:
                    f(block=j)
                pending_proj.pop(0)()
            pending_bc.clear()

    return nc


_PROGRAM = None


def _get_program():
    global _PROGRAM
    if _PROGRAM is None:
        _PROGRAM = _build_program()
    return _PROGRAM


def _bf16(a):
    import ml_dtypes

    return np.asarray(a, dtype=np.float32).astype(ml_dtypes.bfloat16)


def _prep_core_inputs(x, W_qkv, b_qkv, heads, batch):
    """Host-side slicing/relayout for one core."""
    cols = np.concatenate([np.arange(h * HD, (h + 1) * HD) for h in heads])
    # q weights/bias pre-scaled by 1/sqrt(C): scores arrive ready for exp
    w_q = W_qkv[:, cols] * SCALE          # [512, 128]
    w_k = W_qkv[:, C + cols]
    w_v = W_qkv[:, 2 * C + cols]
    # both heads packed in one M=128 projection (rows of qT = stacked heads)
    w_q = np.ascontiguousarray(w_q.reshape(NCJ, 128, HPC * HD).transpose(1, 0, 2))
    w_k = np.ascontiguousarray(w_k.reshape(NCJ, 128, HPC * HD).transpose(1, 0, 2))
    w_v = np.ascontiguousarray(
        w_v.reshape(NCJ, 128, HPC * HD).transpose(1, 0, 2))

    b_q = (b_qkv[cols] * SCALE).reshape(128, 1).astype(np.float32)
    b_k = b_qkv[C + cols].reshape(128, 1).astype(np.float32)
    xt = np.ascontiguousarray(
        x[batch].T.reshape(NCJ, 128, QB, 512).transpose(2, 1, 0, 3))
    return {
        "xt": _bf16(xt),
        "w_q": _bf16(w_q),
        "w_k": _bf16(w_k),
        "w_v": _bf16(w_v),
        "b_q": b_q,
        "b_k": b_k,
    }


def _core_w_o(W_out, heads):
    rows = np.concatenate([np.arange(h * HD, (h + 1) * HD) for h in heads])
    w = np.zeros((128, HPC, C), dtype=np.float32)
    w[0:HD] = W_out[rows].reshape(HPC, HD, C).transpose(1, 0, 2)
    return _bf16(w)


def kernel(x, W_qkv, b_qkv, W_out, b_out):
    x = np.asarray(x, dtype=np.float32)
    W_qkv = np.asarray(W_qkv, dtype=np.float32)
    b_qkv = np.asarray(b_qkv, dtype=np.float32)
    W_out = np.asarray(W_out, dtype=np.float32)
    b_out = np.asarray(b_out, dtype=np.float32)

    nc = _get_program()
    in_maps = []
    for c in range(NCORES):
        batch, hp = c // 4, c % 4
        heads = [2 * hp, 2 * hp + 1]
        im = _prep_core_inputs(x, W_qkv, b_qkv, heads, batch)
        im["w_o"] = _core_w_o(W_out, heads)
        in_maps.append(im)

    res = run_bass_kernel_spmd(nc, in_maps, core_ids=list(range(NCORES)))

    # v-bias commutes: softmax rows sum to 1, so (P @ (V + 1 b_v)) @ W_o
    # = P@V@W_o + b_v@W_o. Add b_v@W_out and b_out once on the host.
    const_row = b_qkv[2 * C:] @ W_out + b_out    # [512]
    out = np.empty((B, N, C), dtype=np.float32)
    for b in range(B):
        acc = res.results[4 * b]["out"].astype(np.float32).copy()
        for c in range(4 * b + 1, 4 * b + 4):
            acc += res.results[c]["out"]
        out[b] = acc + const_row
    return out


# revision 51
# speedup vs baseline: 1.0096x; 1.0096x over previous
"""Multi-head attention (B=2, N=4096, C=512, H=8) on 8 TRN2 NeuronCores.

Sharding: core c handles batch c//4 and heads {2*(c%4), 2*(c%4)+1}
(data parallel over batch, tensor parallel over heads). Each core
computes its 2 heads' attention plus a partial output projection;
the host sums the 4 partials per batch and adds the bias terms
(b_out and b_v @ W_out, which commutes past softmax-weighted sums).

Per-core compute (matmul operands bf16, all accumulation f32):
  xT   host-pretransposed, streamed in per 512-token block
  kT   both heads stacked on K=128 (one M=128 projection per block)
  qT   both heads stacked on K=128 (one M=128 projection per block);
       q weights/biases host-scaled by 1/sqrt(C) so scores arrive
       pre-scaled and exp runs with scale=1
  v    projected straight into PV-operand layout (lhsT = xT block),
       with a memset ones column so P.T@[v|1] yields softmax sums free
  attention, one flat software-pipelined stream over (qb, kb):
     scoresT for BOTH heads concurrently via PE row tiling: two K=64
     matmuls at tile_position (0,0) and (64,0) sharing the stacked qT
     stream -> PSUM ks [128, 2, 512] f32 (one bank per head); the two
     MMs start ~6ns apart and run on disjoint row groups, so a head's
     scores cost half of the zero-padded K=128 formulation
     expT = Exp(scoresT) -> SBUF bf16 (one wide ACT op, FD=1024); the
     scalar engine is the bottleneck of the whole kernel (~262us of
     exp at 1 elem/cycle/lane; nothing else can compute exp -- custom
     DVE ops are unsupported by this walrus), so everything else is
     arranged to keep its stream gapless
     accum v_aug.T @ expT per head over kb -> PSUM po_h [65, 512];
     PV for (kb, h) lags the scores by 3+h flat steps and the queue
     DRAINS ACROSS qb boundaries, so the PE chews leftover PVs while
     the first exp of the new qb frees the scores ring
  deferred tails (per qb, running during the next qb so the in-order
  engine FIFOs never head-block the hot stream):
     po evac copy fires INLINE right after that head's last PV (frees
     the bank before the next qb's PV rotates onto it); g4: [1,512]
     iterative reciprocal of the sums row (DVE); g12: DRAM round trip
     + SWDGE broadcast-read across 64 partitions, normalize (the DMAs
     ride the otherwise-idle GPSIMD queue); g15/17/19/21: output
     projection (both heads accumulated, K=128 zero-padded) + store
     For the LAST qb the two reciprocals would sit on the exit
     critical path, so both sums rows round-trip through DRAM into a
     [128, 8] relayout for one ~0.2us reciprocal, the chain rides the
     by-then-idle sync queue, and ~35 dummy matmuls keep the HAM
     clock gate at 2.4 GHz through the round trip (else the
     projections run cold at 1.2 GHz)
  qkv production for block t is interleaved at groups 4(t-1)/4(t-1)+2
  of qb 0 (k+v01 then v23+q), sharing the projection PSUM tag.

PSUM: ks 2 bufs x 2 banks + po/warmt 2 bufs x 1 bank + production/
proj "pp" 2 bufs x 1 bank = 8 banks exactly.
"""

import numpy as np

import concourse.bass as bass
import concourse.mybir as mybir
import concourse.tile as tile
from concourse.bass_utils import run_bass_kernel_spmd
from concourse.tile_rust import add_dep_helper
from concourse.vector_clock import ScopedClock

F32 = mybir.dt.float32
BF16 = mybir.dt.bfloat16
AF = mybir.ActivationFunctionType

B, N, C, H = 2, 4096, 512, 8
HD = C // H          # 64
HPC = H // 4         # 2 heads per core
NCORES = 8
NT = N // 128        # 32 key chunks
NCJ = C // 128       # 4 contraction chunks
QB = N // 512        # 8 query blocks
SCALE = 1.0 / float(np.sqrt(C))


def _patch_tail_drain():
    """This walrus build caps sync waits at 1 per non-EventSemaphore
    instruction (2 for EventSemaphore); the stock TileContext tail-drain
    attaches every outstanding wait to one Drain, and the scheduler can
    leave >1 wait on regular instructions. Spill extras onto fresh
    same-engine nops inserted just before the over-subscribed one."""
    if getattr(tile.TileContext, "_drain_patched", False):
        return

    def _spill_excess_waits(nc):
        for fn in nc.m.functions:
            for bb in fn.blocks:
                insts = bb.instructions
                i = 0
                while i < len(insts):
                    inst = insts[i]
                    si = inst.sync_info
                    cap = 2 if isinstance(inst, mybir.InstEventSemaphore) else 1
                    if si is None or len(si.on_wait) <= cap:
                        i += 1
                        continue
                    extra = list(si.on_wait[cap:])
                    si.on_wait[:] = si.on_wait[:cap]
                    for w in extra:
                        nop = nc.engines[inst.engine].nop(
                            hint="wait_spill", nofuse=True
                        )
                        cur = nc.cur_bb.bb.instructions
                        cur.remove(nop.ins)
                        if nop.ins.sync_info is None:
                            nop.ins.sync_info = mybir.SyncInfo(
                                on_update=[], on_wait=[]
                            )
                        nop.ins.sync_info.on_wait.append(w)
                        insts.insert(i, nop.ins)
                        i += 1
                    i += 1

    def _drain_and_barrier(self, tick_clock, wait_clock):
        nc = self.nc
        drain_inst = nc.sync.drain()
        wait_clock.add_sem_waits(
            drain_inst.ins, ScopedClock({None: tick_clock.global_clock})
        )
        nc.all_engine_barrier()
        assert self.sems is not None
        popped = nc._tile_sem_poison_stack.pop()
        assert popped is self._sem_poison
        nc.clear_and_free_semaphores(list(self.sems.allocated().values()))
        nc.all_engine_barrier()
        _spill_excess_waits(nc)

    tile.TileContext._drain_and_barrier = _drain_and_barrier
    tile.TileContext._drain_patched = True


def _build_program():
    _patch_tail_drain()
    nc = bass.Bass()

    xt = nc.dram_tensor("xt", [QB, 128, NCJ, 512], BF16, kind="ExternalInput")
    # host-prearranged weight layouts (see kernel() below); both heads are
    # packed into one M=128 projection for q (rows 0:64 = h0, 64:128 = h1),
    # matching the stacked kT rows so the row-tiled scores matmuls read
    # aligned 64-partition slices of one qT tile
    w_q = nc.dram_tensor("w_q", [128, NCJ, 128], BF16, kind="ExternalInput")
    w_k = nc.dram_tensor("w_k", [128, NCJ, 128], BF16, kind="ExternalInput")
    w_v = nc.dram_tensor("w_v", [128, NCJ, HPC * HD], BF16, kind="ExternalInput")
    w_o = nc.dram_tensor("w_o", [128, HPC, C], BF16, kind="ExternalInput")
    b_q = nc.dram_tensor("b_q", [128, 1], F32, kind="ExternalInput")
    b_k = nc.dram_tensor("b_k", [128, 1], F32, kind="ExternalInput")
    out = nc.dram_tensor("out", [N, C], F32, kind="ExternalOutput")
    scratch = nc.dram_tensor("scratch", [QB * HPC + 2, 512], F32)

    from contextlib import ExitStack

    with tile.TileContext(nc) as tc, ExitStack() as ctx:
        # PE warmup: ~4us of back-to-back dummy matmuls during the initial
        # DMA wait so the HAM clock gate reaches K=8/8 (2.4 GHz) before the
        # first real matmul; garbage operand values are fine.
        const = ctx.enter_context(tc.tile_pool(name="const", bufs=1))
        w_q_sb = const.tile([128, NCJ, 128], BF16)
        w_k_sb = const.tile([128, NCJ, 128], BF16)
        w_v_sb = const.tile([128, NCJ, HPC * HD], BF16)
        w_o_sb = const.tile([128, HPC, C], BF16)
        b_q_sb = const.tile([128, 1], F32)
        b_k_sb = const.tile([128, 1], F32)

        persist = ctx.enter_context(tc.tile_pool(name="persist", bufs=1))
        # qT/kT hold BOTH heads stacked (rows 0:64 = h0, 64:128 = h1); the
        # scores row-tiling selects the head via the K=64 partition slice
        qT = persist.tile([128, N], BF16)
        kT = persist.tile([128, N], BF16)
        # [tokens, kb, head, 128]: dims at 0:64, ones at 64 (from memset)
        v_nat = persist.tile([128, NT, HPC, 128], BF16)

        # ---- fused pipeline: qkv production interleaved into attention ----
        with (
            tc.tile_pool(name="xTp", bufs=1) as xTp,
            tc.tile_pool(name="oTp", bufs=1) as oTp,
            tc.tile_pool(name="expp", bufs=6) as expp,
            tc.tile_pool(name="posb", bufs=3) as posb,
            tc.tile_pool(name="recipp", bufs=3) as recipp,
            tc.tile_pool(name="bcsb", bufs=3) as bcsb,
            tc.tile_pool(name="ostage", bufs=4) as ostage,
            tc.tile_pool(name="ps_s", bufs=2, space="PSUM") as ps_s,
            tc.tile_pool(name="ps_o", bufs=2, space="PSUM") as ps_o,
            tc.tile_pool(name="ps_p", bufs=2, space="PSUM") as ps_p,
        ):
            # oT zero-padded to K=128 (rows 64:128 stay 0; w_o rows there are
            # host-zeroed) so the projection avoids the K=64/M=128 slow path
            oT = oTp.tile([128, HPC, N], BF16)
            xT = xTp.tile([128, NCJ, N], BF16)
            # DMA waits are precise per-transfer semaphores; what gates the
            # head is TRANSFER order. Small weights go first on the sync
            # queue while xT block 0 streams per-cj IN PARALLEL on the
            # gpsimd queue (subtile deps let the first k-matmul start
            # after w_k + just the cj0 slice). The big oT memset is
            # emitted after the xT loads so it doesn't block the gpsimd
            # descriptor stream (it's only needed by ~55us in).
            nc.sync.dma_start(out=w_k_sb, in_=w_k[:])
            nc.sync.dma_start(out=b_k_sb, in_=b_k[:])
            nc.sync.dma_start(out=w_q_sb, in_=w_q[:])
            nc.sync.dma_start(out=b_q_sb, in_=b_q[:])
            nc.sync.dma_start(out=w_v_sb, in_=w_v[:])
            nc.sync.dma_start(out=w_o_sb, in_=w_o[:])
            for cj in range(NCJ):
                nc.gpsimd.dma_start(
                    out=xT[:, cj, 0:512], in_=xt[0][:, cj, :]
                )
            for tb in range(1, QB):
                tsl = slice(tb * 512, (tb + 1) * 512)
                nc.gpsimd.dma_start(out=xT[:, :, tsl], in_=xt[tb])
            nc.gpsimd.memset(oT[HD:128, :, :], 0.0)
            # only the ones column needs initializing: PV reads cols 0:65
            nc.vector.memset(v_nat[:, :, :, HD:HD + 1], 1.0)

            last_pe = {"inst": None}

            def pe_keepwarm(n):
                """Dummy matmuls to hold the HAM clock gate at K=8/8.
                Reading w_k_sb pins them after its DMA (so the PE warms
                during the load, right before the first real matmul);
                chaining off last_pe pins tail bursts after the last PV."""
                wt = ps_o.tile([128, 512], F32, name="warmt", tag="po")
                for _ in range(n):
                    mm = nc.tensor.matmul(
                        wt, lhsT=w_k_sb[:, 0, :], rhs=qT[:, 0:512],
                        start=True, stop=True,
                    )
                    if last_pe["inst"] is not None:
                        add_dep_helper(
                            mm.ins, last_pe["inst"], sync=False,
                            reason="keepwarm pinned to tail",
                        )
                    last_pe["inst"] = mm.ins

            def vnat_kb(kb):
                ksl = slice(kb * 128, (kb + 1) * 128)
                pv_ = ps_p.tile([128, HPC * HD], F32, tag="pp", name="pv_")
                for cj in range(NCJ):
                    nc.tensor.matmul(
                        pv_,
                        lhsT=xT[:, cj, ksl],
                        rhs=w_v_sb[:, cj, :],
                        start=(cj == 0),
                        stop=(cj == NCJ - 1),
                    )
                nc.vector.tensor_copy(
                    out=v_nat[:, kb, :, 0:HD],
                    in_=pv_.rearrange("p (h d) -> p h d", h=HPC),
                )

            def production(tb, part=None):
                """qkv projections for one 512-token block; psum via the
                shared 'pp' tag (temporally disjoint from proj use).
                parts: 0=k, 1=v01, 2=v23, 3=q (one per group, spread so
                qb0's per-group PE burden stays under the ACT pace);
                "head": k+q then all v (block-0 path)."""
                tsl = slice(tb * 512, (tb + 1) * 512)
                if part in (0, None, "head"):
                    pk = ps_p.tile([128, 512], F32, tag="pp", name="pk")
                    for cj in range(NCJ):
                        nc.tensor.matmul(
                            pk,
                            lhsT=w_k_sb[:, cj, :],
                            rhs=xT[:, cj, tsl],
                            start=(cj == 0),
                            stop=(cj == NCJ - 1),
                        )
                    nc.vector.tensor_scalar_add(
                        out=kT[:, tsl], in0=pk, scalar1=b_k_sb
                    )
                if part in (1, None):
                    for kb in range(tb * 4, tb * 4 + 2):
                        vnat_kb(kb)
                if part in (2, None):
                    for kb in range(tb * 4 + 2, tb * 4 + 4):
                        vnat_kb(kb)
                if part in (3, None, "head"):
                    pm = ps_p.tile([128, 512], F32, tag="pp", name="pm")
                    for cj in range(NCJ):
                        nc.tensor.matmul(
                            pm,
                            lhsT=w_q_sb[:, cj, :],
                            rhs=xT[:, cj, tsl],
                            start=(cj == 0),
                            stop=(cj == NCJ - 1),
                        )
                    nc.vector.tensor_scalar_add(
                        out=qT[:, tsl], in0=pm, scalar1=b_q_sb
                    )
                if part == "head":
                    for kb in range(tb * 4, tb * 4 + 4):
                        vnat_kb(kb)

            # block 0: k and q first (the first scores pair needs only
            # those), v afterwards -- the first exp starts ~3us sooner
            production(0, part="head")
            last_exp = {"inst": None}
            last_pe = {"inst": None}
            evac_by = {}        # (qb, h) -> copy thunk, fired inline right
                                # after that head's LAST PV (so the bank is
                                # freed before the next qb rotates onto it,
                                # and never copied before the accum is done)
            pending_recip = []  # flushed @g4 of the following qb (DVE)
            pending_bc = []     # flushed @g12 (SWDGE bcast + DVE normalize)
            pending_proj = []   # flushed @g15/17/19/21 (PE matmuls)

            fin = {"wr0": [], "wr1": None}

            def make_tail(po, h, qsl, u2, final=False):
                """Tail queue discipline: the evac COPY frees the PSUM bank
                (emitted for both heads before either fat reciprocal so the
                next qb's PV never waits behind a 3.4us recip in the DVE
                FIFO); the scratch write rides the GPSIMD queue (its only
                other traffic is this tail) so the cross-engine wait never
                head-blocks the out-store DMA queue."""
                state = {}

                def evac_copy():
                    ps = posb.tile([HD + 1, 512], F32, name="ps")
                    nc.vector.tensor_copy(out=ps, in_=po)
                    state["ps"] = ps

                def evac_recip():
                    if final:
                        # last qb: the 3.4us/head one-lane reciprocal sits
                        # on the exit critical path -- spread BOTH heads'
                        # sums across 128 partitions via a DRAM relayout
                        # and do one [128, 8] reciprocal (~0.2us)
                        fin["wr0"].append(nc.sync.dma_start(
                            out=scratch[u2:u2 + 1, :],
                            in_=state["ps"][HD:HD + 1, :],
                        ))
                        if len(fin["wr0"]) == 2:
                            base = (QB * HPC - HPC) * 512
                            rs = recipp.tile(
                                [128, 8], F32, name="rs", tag="rs"
                            )
                            rd0 = nc.sync.dma_start(
                                out=rs,
                                in_=bass.AP(
                                    tensor=scratch, offset=base,
                                    ap=[[8, 128], [1, 8]],
                                ),
                            )
                            for w in fin["wr0"]:
                                add_dep_helper(
                                    rd0.ins, w.ins, sync=True,
                                    reason="final sums relayout RAW",
                                )
                            rr = recipp.tile(
                                [128, 8], F32, name="rr", tag="rr"
                            )
                            nc.vector.reciprocal(out=rr, in_=rs)
                            fin["wr1"] = nc.sync.dma_start(
                                out=bass.AP(
                                    tensor=scratch,
                                    offset=(QB * HPC) * 512,
                                    ap=[[8, 128], [1, 8]],
                                ),
                                in_=rr,
                            )
                        return
                    rt = recipp.tile([1, 512], F32, name="rt")
                    nc.vector.reciprocal(
                        out=rt, in_=state["ps"][HD:HD + 1, :]
                    )
                    # round-trip through DRAM to broadcast across
                    # partitions (no on-chip partition-broadcast path)
                    state["wr"] = nc.gpsimd.dma_start(
                        out=scratch[u2:u2 + 1, :], in_=rt
                    )

                def bcmult(block=None):
                    bc = state.get("bc")
                    if bc is None:
                        src_row = (QB * HPC + h) if final else u2
                        bc = bcsb.tile([HD, 512], F32, name="bc")
                        dma_eng = nc.sync if final else nc.gpsimd
                        rd = dma_eng.dma_start(
                            out=bc,
                            in_=bass.AP(
                                tensor=scratch, offset=src_row * 512,
                                ap=[[0, HD], [1, 512]],
                            ),
                        )
                        add_dep_helper(
                            rd.ins,
                            (fin["wr1"] if final else state["wr"]).ins,
                            sync=True,
                            reason="recip broadcast RAW",
                        )
                        state["bc"] = bc
                    csl = slice(0, 512) if block is None else slice(
                        block * 128, (block + 1) * 128
                    )
                    osl = slice(qsl.start + csl.start, qsl.start + csl.stop)
                    nc.vector.tensor_mul(
                        out=oT[0:HD, h, osl],
                        in0=state["ps"][0:HD, csl], in1=bc[:, csl],
                    )
                return evac_copy, evac_recip, bcmult

            def make_proj(qb, j):
                def proj():
                    q0 = qb * 512 + j * 128
                    pp = ps_p.tile([128, C], F32, tag="pp", name="pp")
                    for h in range(HPC):
                        mm = nc.tensor.matmul(
                            pp,
                            lhsT=oT[:, h, q0:q0 + 128],
                            rhs=w_o_sb[:, h, :],
                            start=(h == 0),
                            stop=(h == HPC - 1),
                        )
                        if h == 0 and last_exp["inst"] is not None:
                            add_dep_helper(
                                mm.ins, last_exp["inst"], sync=False,
                                reason="proj after normalize really done",
                            )
                    ot = ostage.tile([128, C], F32, name="ot")
                    nc.vector.tensor_copy(out=ot, in_=pp)
                    nc.sync.dma_start(out=out[q0:q0 + 128, :], in_=ot)
                return proj

            def s_pair(qb, kb):
                """Both heads' scoresT for one key chunk, concurrently on
                the PE via row tiling (K=64 each, shared qT stream)."""
                qsl = slice(qb * 512, (qb + 1) * 512)
                ksl = slice(kb * 128, (kb + 1) * 128)
                ks = ps_s.tile([128, HPC, 512], F32, name="ks")
                for h in range(HPC):
                    nc.tensor.matmul(
                        ks[:, h, :],
                        lhsT=kT[h * HD:(h + 1) * HD, ksl],
                        rhs=qT[h * HD:(h + 1) * HD, qsl],
                        start=True,
                        stop=True,
                        tile_position=(h * HD, 0),
                    )
                return ks

            # flat (qb, kb) pipeline: the scores skew AND the lagged PV
            # queue carry across qb boundaries so the PE/ACT streams never
            # drain (the PE chews leftover PVs while the first exp of the
            # new qb frees the scores ring)
            flat = [(qb, kb) for qb in range(QB) for kb in range(NT)]
            po_tiles = {}
            pending_pv = []   # (i, qb, kb, h, et); lag 3+h flat steps

            def pv_one(qb_, kb_, h_, et_):
                if (qb_, h_) not in po_tiles:
                    po_tiles[(qb_, h_)] = ps_o.tile(
                        [HD + 1, 512], F32, name="po"
                    )
                mm = nc.tensor.matmul(
                    po_tiles[(qb_, h_)],
                    lhsT=v_nat[:, kb_, h_, 0:HD + 1],
                    rhs=et_[:, h_, :],
                    start=(kb_ == 0),
                    stop=(kb_ == NT - 1),
                )
                last_pe["inst"] = mm.ins
                if kb_ == NT - 1 and (qb_, h_) in evac_by:
                    evc, rec = evac_by.pop((qb_, h_))
                    evc()
                    pending_recip.append(rec)

            pend = s_pair(*flat[0])
            for i, (qb, kb) in enumerate(flat):
                ks = pend
                pend = s_pair(*flat[i + 1]) if i + 1 < len(flat) else None
                g = kb
                if g == 4:
                    for r in pending_recip:
                        r()
                    pending_recip.clear()
                # qb0 produces only k+v of blocks 1..7 (every kb sweep
                # needs them); each block's Q projection is needed only
                # when ITS query sweep starts, so it is deferred to the
                # preceding qb -- this sheds ~6.6us of PE work from qb0,
                # whose production load exceeds the ACT-pace slack
                if qb == 0 and g % 2 == 0 and g // 4 + 1 < QB:
                    production(g // 4 + 1, part=g % 4)
                    if g % 4 == 0:
                        production(g // 4 + 1, part=1)
                if qb == 0 and g == 28:
                    production(1, part=3)
                if 1 <= qb < QB - 1 and g == 24:
                    production(qb + 1, part=3)
                if g == 12:
                    for f in pending_bc:
                        f()
                    pending_bc.clear()
                elif g in (15, 17, 19, 21) and pending_proj:
                    pending_proj.pop(0)()
                et = expp.tile([128, HPC, 512], BF16)
                exp_bi = nc.scalar.activation(
                    out=et, in_=ks, func=AF.Exp, scale=1.0
                )
                last_exp["inst"] = exp_bi.ins

                for h in range(HPC):
                    pending_pv.append((i, qb, kb, h, et))
                while pending_pv and i - pending_pv[0][0] >= 3 + pending_pv[0][3]:
                    pv_one(*pending_pv.pop(0)[1:])
                if kb == NT - 1:
                    qsl = slice(qb * 512, (qb + 1) * 512)
                    for h in range(HPC):
                        evc, rec, bcm = make_tail(
                            po_tiles[(qb, h)], h, qsl, qb * HPC + h,
                            final=(qb == QB - 1),
                        )
                        evac_by[(qb, h)] = (evc, rec)
                        pending_bc.append(bcm)
                    for j in range(4):
                        pending_proj.append(make_proj(qb, j))
            while pending_pv:
                pv_one(*pending_pv.pop(0)[1:])
            # --- final-qb tail: keep the PE warm through the reciprocal
            # round trip (else HAM re-throttles and the projections run at
            # 1.2 GHz), and interleave per-128-block normalize with the
            # projections to shorten the serial chain ---
            for r in pending_recip:
                r()
            pending_recip.clear()
            pe_keepwarm(46)
            for j in range(4):
                for f in pending_bc:
                    f(block=j)
                pending_proj.pop(0)()
            pending_bc.clear()

    return nc


_PROGRAM = None


def _get_program():
    global _PROGRAM
    if _PROGRAM is None:
        _PROGRAM = _build_program()
    return _PROGRAM


def _bf16(a):
    import ml_dtypes

    return np.asarray(a, dtype=np.float32).astype(ml_dtypes.bfloat16)


def _prep_core_inputs(x, W_qkv, b_qkv, heads, batch):
    """Host-side slicing/relayout for one core."""
    cols = np.concatenate([np.arange(h * HD, (h + 1) * HD) for h in heads])
    # q weights/bias pre-scaled by 1/sqrt(C): scores arrive ready for exp
    w_q = W_qkv[:, cols] * SCALE          # [512, 128]
    w_k = W_qkv[:, C + cols]
    w_v = W_qkv[:, 2 * C + cols]
    # both heads packed in one M=128 projection (rows of qT = stacked heads)
    w_q = np.ascontiguousarray(w_q.reshape(NCJ, 128, HPC * HD).transpose(1, 0, 2))
    w_k = np.ascontiguousarray(w_k.reshape(NCJ, 128, HPC * HD).transpose(1, 0, 2))
    w_v = np.ascontiguousarray(
        w_v.reshape(NCJ, 128, HPC * HD).transpose(1, 0, 2))

    b_q = (b_qkv[cols] * SCALE).reshape(128, 1).astype(np.float32)
    b_k = b_qkv[C + cols].reshape(128, 1).astype(np.float32)
    xt = np.ascontiguousarray(
        x[batch].T.reshape(NCJ, 128, QB, 512).transpose(2, 1, 0, 3))
    return {
        "xt": _bf16(xt),
        "w_q": _bf16(w_q),
        "w_k": _bf16(w_k),
        "w_v": _bf16(w_v),
        "b_q": b_q,
        "b_k": b_k,
    }


def _core_w_o(W_out, heads):
    rows = np.concatenate([np.arange(h * HD, (h + 1) * HD) for h in heads])
    w = np.zeros((128, HPC, C), dtype=np.float32)
    w[0:HD] = W_out[rows].reshape(HPC, HD, C).transpose(1, 0, 2)
    return _bf16(w)


def kernel(x, W_qkv, b_qkv, W_out, b_out):
    x = np.asarray(x, dtype=np.float32)
    W_qkv = np.asarray(W_qkv, dtype=np.float32)
    b_qkv = np.asarray(b_qkv, dtype=np.float32)
    W_out = np.asarray(W_out, dtype=np.float32)
    b_out = np.asarray(b_out, dtype=np.float32)

    nc = _get_program()
    in_maps = []
    for c in range(NCORES):
        batch, hp = c // 4, c % 4
        heads = [2 * hp, 2 * hp + 1]
        im = _prep_core_inputs(x, W_qkv, b_qkv, heads, batch)
        im["w_o"] = _core_w_o(W_out, heads)
        in_maps.append(im)

    res = run_bass_kernel_spmd(nc, in_maps, core_ids=list(range(NCORES)))

    # v-bias commutes: softmax rows sum to 1, so (P @ (V + 1 b_v)) @ W_o
    # = P@V@W_o + b_v@W_o. Add b_v@W_out and b_out once on the host.
    const_row = b_qkv[2 * C:] @ W_out + b_out    # [512]
    out = np.empty((B, N, C), dtype=np.float32)
    for b in range(B):
        acc = res.results[4 * b]["out"].astype(np.float32).copy()
        for c in range(4 * b + 1, 4 * b + 4):
            acc += res.results[c]["out"]
        out[b] = acc + const_row
    return out


# revision 52
# speedup vs baseline: 1.0166x; 1.0069x over previous
"""Multi-head attention (B=2, N=4096, C=512, H=8) on 8 TRN2 NeuronCores.

Sharding: core c handles batch c//4 and heads {2*(c%4), 2*(c%4)+1}
(data parallel over batch, tensor parallel over heads). Each core
computes its 2 heads' attention plus a partial output projection;
the host sums the 4 partials per batch and adds the bias terms
(b_out and b_v @ W_out, which commutes past softmax-weighted sums).

Per-core compute (matmul operands bf16, all accumulation f32):
  xT   host-pretransposed, streamed in per 512-token block
  kT   both heads stacked on K=128 (one M=128 projection per block)
  qT   both heads stacked on K=128 (one M=128 projection per block);
       q weights/biases host-scaled by 1/sqrt(C) so scores arrive
       pre-scaled and exp runs with scale=1
  v    projected straight into PV-operand layout (lhsT = xT block),
       with a memset ones column so P.T@[v|1] yields softmax sums free
  attention, one flat software-pipelined stream over (qb, kb):
     scoresT for BOTH heads concurrently via PE row tiling: two K=64
     matmuls at tile_position (0,0) and (64,0) sharing the stacked qT
     stream -> PSUM ks [128, 2, 512] f32 (one bank per head); the two
     MMs start ~6ns apart and run on disjoint row groups, so a head's
     scores cost half of the zero-padded K=128 formulation
     expT = Exp(scoresT) -> SBUF bf16 (one wide ACT op, FD=1024); the
     scalar engine is the bottleneck of the whole kernel (~262us of
     exp at 1 elem/cycle/lane; nothing else can compute exp -- custom
     DVE ops are unsupported by this walrus), so everything else is
     arranged to keep its stream gapless
     accum v_aug.T @ expT per head over kb -> PSUM po_h [65, 512];
     PV for (kb, h) lags the scores by 3+h flat steps and the queue
     DRAINS ACROSS qb boundaries, so the PE chews leftover PVs while
     the first exp of the new qb frees the scores ring
  deferred tails (per qb, running during the next qb so the in-order
  engine FIFOs never head-block the hot stream):
     po evac copy fires INLINE right after that head's last PV (frees
     the bank before the next qb's PV rotates onto it); g4: [1,512]
     iterative reciprocal of the sums row (DVE); g12: DRAM round trip
     + SWDGE broadcast-read across 64 partitions, normalize (the DMAs
     ride the otherwise-idle GPSIMD queue); g15/17/19/21: output
     projection (both heads accumulated, K=128 zero-padded) + store
     For the LAST qb the two reciprocals would sit on the exit
     critical path, so both sums rows round-trip through DRAM into a
     [128, 8] relayout for one ~0.2us reciprocal, the chain rides the
     by-then-idle sync queue, and ~35 dummy matmuls keep the HAM
     clock gate at 2.4 GHz through the round trip (else the
     projections run cold at 1.2 GHz)
  qkv production for block t is interleaved at groups 4(t-1)/4(t-1)+2
  of qb 0 (k+v01 then v23+q), sharing the projection PSUM tag.

PSUM: ks 2 bufs x 2 banks + po/warmt 2 bufs x 1 bank + production/
proj "pp" 2 bufs x 1 bank = 8 banks exactly.
"""

import numpy as np

import concourse.bass as bass
import concourse.mybir as mybir
import concourse.tile as tile
from concourse.bass_utils import run_bass_kernel_spmd
from concourse.tile_rust import add_dep_helper
from concourse.vector_clock import ScopedClock

F32 = mybir.dt.float32
BF16 = mybir.dt.bfloat16
AF = mybir.ActivationFunctionType

B, N, C, H = 2, 4096, 512, 8
HD = C // H          # 64
HPC = H // 4         # 2 heads per core
NCORES = 8
NT = N // 128        # 32 key chunks
NCJ = C // 128       # 4 contraction chunks
QB = N // 512        # 8 query blocks
SCALE = 1.0 / float(np.sqrt(C))


def _patch_tail_drain():
    """This walrus build caps sync waits at 1 per non-EventSemaphore
    instruction (2 for EventSemaphore); the stock TileContext tail-drain
    attaches every outstanding wait to one Drain, and the scheduler can
    leave >1 wait on regular instructions. Spill extras onto fresh
    same-engine nops inserted just before the over-subscribed one."""
    if getattr(tile.TileContext, "_drain_patched", False):
        return

    def _spill_excess_waits(nc):
        for fn in nc.m.functions:
            for bb in fn.blocks:
                insts = bb.instructions
                i = 0
                while i < len(insts):
                    inst = insts[i]
                    si = inst.sync_info
                    cap = 2 if isinstance(inst, mybir.InstEventSemaphore) else 1
                    if si is None or len(si.on_wait) <= cap:
                        i += 1
                        continue
                    extra = list(si.on_wait[cap:])
                    si.on_wait[:] = si.on_wait[:cap]
                    for w in extra:
                        nop = nc.engines[inst.engine].nop(
                            hint="wait_spill", nofuse=True
                        )
                        cur = nc.cur_bb.bb.instructions
                        cur.remove(nop.ins)
                        if nop.ins.sync_info is None:
                            nop.ins.sync_info = mybir.SyncInfo(
                                on_update=[], on_wait=[]
                            )
                        nop.ins.sync_info.on_wait.append(w)
                        insts.insert(i, nop.ins)
                        i += 1
                    i += 1

    def _drain_and_barrier(self, tick_clock, wait_clock):
        nc = self.nc
        drain_inst = nc.sync.drain()
        wait_clock.add_sem_waits(
            drain_inst.ins, ScopedClock({None: tick_clock.global_clock})
        )
        nc.all_engine_barrier()
        assert self.sems is not None
        popped = nc._tile_sem_poison_stack.pop()
        assert popped is self._sem_poison
        nc.clear_and_free_semaphores(list(self.sems.allocated().values()))
        nc.all_engine_barrier()
        _spill_excess_waits(nc)

    tile.TileContext._drain_and_barrier = _drain_and_barrier
    tile.TileContext._drain_patched = True


def _build_program():
    _patch_tail_drain()
    nc = bass.Bass()

    xt = nc.dram_tensor("xt", [QB, 128, NCJ, 512], BF16, kind="ExternalInput")
    # host-prearranged weight layouts (see kernel() below); both heads are
    # packed into one M=128 projection for q (rows 0:64 = h0, 64:128 = h1),
    # matching the stacked kT rows so the row-tiled scores matmuls read
    # aligned 64-partition slices of one qT tile
    w_q = nc.dram_tensor("w_q", [128, NCJ, 128], BF16, kind="ExternalInput")
    w_k = nc.dram_tensor("w_k", [128, NCJ, 128], BF16, kind="ExternalInput")
    w_v = nc.dram_tensor("w_v", [128, NCJ, HPC * HD], BF16, kind="ExternalInput")
    w_o = nc.dram_tensor("w_o", [128, HPC, C], BF16, kind="ExternalInput")
    b_q = nc.dram_tensor("b_q", [128, 1], F32, kind="ExternalInput")
    b_k = nc.dram_tensor("b_k", [128, 1], F32, kind="ExternalInput")
    out = nc.dram_tensor("out", [N, C], F32, kind="ExternalOutput")
    scratch = nc.dram_tensor("scratch", [QB * HPC + 2, 512], F32)

    from contextlib import ExitStack

    with tile.TileContext(nc) as tc, ExitStack() as ctx:
        # PE warmup: ~4us of back-to-back dummy matmuls during the initial
        # DMA wait so the HAM clock gate reaches K=8/8 (2.4 GHz) before the
        # first real matmul; garbage operand values are fine.
        const = ctx.enter_context(tc.tile_pool(name="const", bufs=1))
        w_q_sb = const.tile([128, NCJ, 128], BF16)
        w_k_sb = const.tile([128, NCJ, 128], BF16)
        w_v_sb = const.tile([128, NCJ, HPC * HD], BF16)
        w_o_sb = const.tile([128, HPC, C], BF16)
        b_q_sb = const.tile([128, 1], F32)
        b_k_sb = const.tile([128, 1], F32)

        persist = ctx.enter_context(tc.tile_pool(name="persist", bufs=1))
        # qT/kT hold BOTH heads stacked (rows 0:64 = h0, 64:128 = h1); the
        # scores row-tiling selects the head via the K=64 partition slice
        qT = persist.tile([128, N], BF16)
        kT = persist.tile([128, N], BF16)
        # [tokens, kb, head, 128]: dims at 0:64, ones at 64 (from memset)
        v_nat = persist.tile([128, NT, HPC, 128], BF16)

        # ---- fused pipeline: qkv production interleaved into attention ----
        with (
            tc.tile_pool(name="xTp", bufs=1) as xTp,
            tc.tile_pool(name="oTp", bufs=1) as oTp,
            tc.tile_pool(name="expp", bufs=6) as expp,
            tc.tile_pool(name="posb", bufs=3) as posb,
            tc.tile_pool(name="recipp", bufs=3) as recipp,
            tc.tile_pool(name="bcsb", bufs=3) as bcsb,
            tc.tile_pool(name="ostage", bufs=4) as ostage,
            tc.tile_pool(name="ps_s", bufs=2, space="PSUM") as ps_s,
            tc.tile_pool(name="ps_o", bufs=2, space="PSUM") as ps_o,
            tc.tile_pool(name="ps_p", bufs=2, space="PSUM") as ps_p,
        ):
            # oT zero-padded to K=128 (rows 64:128 stay 0; w_o rows there are
            # host-zeroed) so the projection avoids the K=64/M=128 slow path
            oT = oTp.tile([128, HPC, N], BF16)
            xT = xTp.tile([128, NCJ, N], BF16)
            # DMA waits are precise per-transfer semaphores; what gates the
            # head is TRANSFER order. Small weights go first on the sync
            # queue while xT block 0 streams per-cj IN PARALLEL on the
            # gpsimd queue (subtile deps let the first k-matmul start
            # after w_k + just the cj0 slice). The big oT memset is
            # emitted after the xT loads so it doesn't block the gpsimd
            # descriptor stream (it's only needed by ~55us in).
            nc.sync.dma_start(out=w_k_sb, in_=w_k[:])
            nc.sync.dma_start(out=b_k_sb, in_=b_k[:])
            nc.sync.dma_start(out=w_q_sb, in_=w_q[:])
            nc.sync.dma_start(out=b_q_sb, in_=b_q[:])
            nc.sync.dma_start(out=w_v_sb, in_=w_v[:])
            nc.sync.dma_start(out=w_o_sb, in_=w_o[:])
            for cj in range(NCJ):
                nc.gpsimd.dma_start(
                    out=xT[:, cj, 0:512], in_=xt[0][:, cj, :]
                )
            for tb in range(1, QB):
                tsl = slice(tb * 512, (tb + 1) * 512)
                nc.gpsimd.dma_start(out=xT[:, :, tsl], in_=xt[tb])
            nc.gpsimd.memset(oT[HD:128, :, :], 0.0)
            # only the ones column needs initializing: PV reads cols 0:65
            nc.vector.memset(v_nat[:, :, :, HD:HD + 1], 1.0)

            last_pe = {"inst": None}

            def pe_keepwarm(n):
                """Dummy matmuls to hold the HAM clock gate at K=8/8.
                Reading w_k_sb pins them after its DMA (so the PE warms
                during the load, right before the first real matmul);
                chaining off last_pe pins tail bursts after the last PV."""
                wt = ps_o.tile([128, 512], F32, name="warmt", tag="po")
                for _ in range(n):
                    mm = nc.tensor.matmul(
                        wt, lhsT=w_k_sb[:, 0, :], rhs=qT[:, 0:512],
                        start=True, stop=True,
                    )
                    if last_pe["inst"] is not None:
                        add_dep_helper(
                            mm.ins, last_pe["inst"], sync=False,
                            reason="keepwarm pinned to tail",
                        )
                    last_pe["inst"] = mm.ins

            def vnat_kb(kb):
                ksl = slice(kb * 128, (kb + 1) * 128)
                pv_ = ps_p.tile([128, HPC * HD], F32, tag="pp", name="pv_")
                for cj in range(NCJ):
                    nc.tensor.matmul(
                        pv_,
                        lhsT=xT[:, cj, ksl],
                        rhs=w_v_sb[:, cj, :],
                        start=(cj == 0),
                        stop=(cj == NCJ - 1),
                    )
                nc.vector.tensor_copy(
                    out=v_nat[:, kb, :, 0:HD],
                    in_=pv_.rearrange("p (h d) -> p h d", h=HPC),
                )

            def production(tb, part=None):
                """qkv projections for one 512-token block; psum via the
                shared 'pp' tag (temporally disjoint from proj use).
                parts: 0=k, 1=v01, 2=v23, 3=q (one per group, spread so
                qb0's per-group PE burden stays under the ACT pace);
                "head": k+q then all v (block-0 path)."""
                tsl = slice(tb * 512, (tb + 1) * 512)
                if part in (0, None, "head"):
                    pk = ps_p.tile([128, 512], F32, tag="pp", name="pk")
                    for cj in range(NCJ):
                        nc.tensor.matmul(
                            pk,
                            lhsT=w_k_sb[:, cj, :],
                            rhs=xT[:, cj, tsl],
                            start=(cj == 0),
                            stop=(cj == NCJ - 1),
                        )
                    nc.vector.tensor_scalar_add(
                        out=kT[:, tsl], in0=pk, scalar1=b_k_sb
                    )
                if part in (1, None):
                    for kb in range(tb * 4, tb * 4 + 2):
                        vnat_kb(kb)
                if part in (2, None):
                    for kb in range(tb * 4 + 2, tb * 4 + 4):
                        vnat_kb(kb)
                if part in (3, None, "head"):
                    pm = ps_p.tile([128, 512], F32, tag="pp", name="pm")
                    for cj in range(NCJ):
                        nc.tensor.matmul(
                            pm,
                            lhsT=w_q_sb[:, cj, :],
                            rhs=xT[:, cj, tsl],
                            start=(cj == 0),
                            stop=(cj == NCJ - 1),
                        )
                    nc.vector.tensor_scalar_add(
                        out=qT[:, tsl], in0=pm, scalar1=b_q_sb
                    )
                if part == "head":
                    for kb in range(tb * 4, tb * 4 + 4):
                        vnat_kb(kb)

            # block 0: k and q first (the first scores pair needs only
            # those), v afterwards -- the first exp starts ~3us sooner
            production(0, part="head")
            last_exp = {"inst": None}
            last_pe = {"inst": None}
            evac_by = {}        # (qb, h) -> copy thunk, fired inline right
                                # after that head's LAST PV (so the bank is
                                # freed before the next qb rotates onto it,
                                # and never copied before the accum is done)
            pending_recip = []  # flushed @g4 of the following qb (DVE)
            pending_bc = []     # flushed @g12 (SWDGE bcast + DVE normalize)
            pending_proj = []   # flushed @g15/17/19/21 (PE matmuls)

            fin = {"wr0": [], "wr1": None}

            def make_tail(po, h, qsl, u2, final=False):
                """Tail queue discipline: the evac COPY frees the PSUM bank
                (emitted for both heads before either fat reciprocal so the
                next qb's PV never waits behind a 3.4us recip in the DVE
                FIFO); the scratch write rides the GPSIMD queue (its only
                other traffic is this tail) so the cross-engine wait never
                head-blocks the out-store DMA queue."""
                state = {}

                def evac_copy():
                    ps = posb.tile([HD + 1, 512], F32, name="ps")
                    nc.vector.tensor_copy(out=ps, in_=po)
                    state["ps"] = ps

                def evac_recip():
                    if final:
                        # last qb: the 3.4us/head one-lane reciprocal sits
                        # on the exit critical path -- spread BOTH heads'
                        # sums across 128 partitions via a DRAM relayout
                        # and do one [128, 8] reciprocal (~0.2us)
                        fin["wr0"].append(nc.sync.dma_start(
                            out=scratch[u2:u2 + 1, :],
                            in_=state["ps"][HD:HD + 1, :],
                        ))
                        if len(fin["wr0"]) == 2:
                            base = (QB * HPC - HPC) * 512
                            rs = recipp.tile(
                                [128, 8], F32, name="rs", tag="rs"
                            )
                            rd0 = nc.sync.dma_start(
                                out=rs,
                                in_=bass.AP(
                                    tensor=scratch, offset=base,
                                    ap=[[8, 128], [1, 8]],
                                ),
                            )
                            for w in fin["wr0"]:
                                add_dep_helper(
                                    rd0.ins, w.ins, sync=True,
                                    reason="final sums relayout RAW",
                                )
                            rr = recipp.tile(
                                [128, 8], F32, name="rr", tag="rr"
                            )
                            nc.vector.reciprocal(out=rr, in_=rs)
                            fin["wr1"] = nc.sync.dma_start(
                                out=bass.AP(
                                    tensor=scratch,
                                    offset=(QB * HPC) * 512,
                                    ap=[[8, 128], [1, 8]],
                                ),
                                in_=rr,
                            )
                        return
                    rt = recipp.tile([1, 512], F32, name="rt")
                    nc.vector.reciprocal(
                        out=rt, in_=state["ps"][HD:HD + 1, :]
                    )
                    # round-trip through DRAM to broadcast across
                    # partitions (no on-chip partition-broadcast path)
                    state["wr"] = nc.gpsimd.dma_start(
                        out=scratch[u2:u2 + 1, :], in_=rt
                    )

                def bcmult(block=None):
                    bc = state.get("bc")
                    if bc is None:
                        src_row = (QB * HPC + h) if final else u2
                        bc = bcsb.tile([HD, 512], F32, name="bc")
                        dma_eng = nc.sync if final else nc.gpsimd
                        rd = dma_eng.dma_start(
                            out=bc,
                            in_=bass.AP(
                                tensor=scratch, offset=src_row * 512,
                                ap=[[0, HD], [1, 512]],
                            ),
                        )
                        add_dep_helper(
                            rd.ins,
                            (fin["wr1"] if final else state["wr"]).ins,
                            sync=True,
                            reason="recip broadcast RAW",
                        )
                        state["bc"] = bc
                    csl = slice(0, 512) if block is None else slice(
                        block * 128, (block + 1) * 128
                    )
                    osl = slice(qsl.start + csl.start, qsl.start + csl.stop)
                    nc.vector.tensor_mul(
                        out=oT[0:HD, h, osl],
                        in0=state["ps"][0:HD, csl], in1=bc[:, csl],
                    )
                return evac_copy, evac_recip, bcmult

            def make_proj(qb, j):
                def proj():
                    q0 = qb * 512 + j * 128
                    pp = ps_p.tile([128, C], F32, tag="pp", name="pp")
                    for h in range(HPC):
                        mm = nc.tensor.matmul(
                            pp,
                            lhsT=oT[:, h, q0:q0 + 128],
                            rhs=w_o_sb[:, h, :],
                            start=(h == 0),
                            stop=(h == HPC - 1),
                        )
                        if h == 0 and last_exp["inst"] is not None:
                            add_dep_helper(
                                mm.ins, last_exp["inst"], sync=False,
                                reason="proj after normalize really done",
                            )
                    ot = ostage.tile([128, C], F32, name="ot")
                    nc.vector.tensor_copy(out=ot, in_=pp)
                    nc.sync.dma_start(out=out[q0:q0 + 128, :], in_=ot)
                return proj

            def s_pair(qb, kb):
                """Both heads' scoresT for one key chunk, concurrently on
                the PE via row tiling (K=64 each, shared qT stream)."""
                qsl = slice(qb * 512, (qb + 1) * 512)
                ksl = slice(kb * 128, (kb + 1) * 128)
                ks = ps_s.tile([128, HPC, 512], F32, name="ks")
                for h in range(HPC):
                    nc.tensor.matmul(
                        ks[:, h, :],
                        lhsT=kT[h * HD:(h + 1) * HD, ksl],
                        rhs=qT[h * HD:(h + 1) * HD, qsl],
                        start=True,
                        stop=True,
                        tile_position=(h * HD, 0),
                    )
                return ks

            # flat (qb, kb) pipeline: the scores skew AND the lagged PV
            # queue carry across qb boundaries so the PE/ACT streams never
            # drain (the PE chews leftover PVs while the first exp of the
            # new qb frees the scores ring)
            flat = [(qb, kb) for qb in range(QB) for kb in range(NT)]
            po_tiles = {}
            pending_pv = []   # (i, qb, kb, h, et); lag 3+h flat steps

            def pv_one(qb_, kb_, h_, et_):
                if (qb_, h_) not in po_tiles:
                    po_tiles[(qb_, h_)] = ps_o.tile(
                        [HD + 1, 512], F32, name="po"
                    )
                mm = nc.tensor.matmul(
                    po_tiles[(qb_, h_)],
                    lhsT=v_nat[:, kb_, h_, 0:HD + 1],
                    rhs=et_[:, h_, :],
                    start=(kb_ == 0),
                    stop=(kb_ == NT - 1),
                )
                last_pe["inst"] = mm.ins
                if kb_ == NT - 1 and (qb_, h_) in evac_by:
                    evc, rec = evac_by.pop((qb_, h_))
                    evc()
                    pending_recip.append(rec)

            pend = s_pair(*flat[0])
            for i, (qb, kb) in enumerate(flat):
                ks = pend
                pend = s_pair(*flat[i + 1]) if i + 1 < len(flat) else None
                g = kb
                if g == 4:
                    for r in pending_recip:
                        r()
                    pending_recip.clear()
                # qb0 produces only k+v of blocks 1..7 (every kb sweep
                # needs them); each block's Q projection is needed only
                # when ITS query sweep starts, so it is deferred to the
                # preceding qb -- this sheds ~6.6us of PE work from qb0,
                # whose production load exceeds the ACT-pace slack
                if qb == 0 and g >= 2 and g % 2 == 0 and (g - 2) // 4 + 1 < QB:
                    production((g - 2) // 4 + 1, part=(g - 2) % 4)
                    if (g - 2) % 4 == 0:
                        production((g - 2) // 4 + 1, part=1)
                if qb == 0 and g == 29:
                    production(1, part=3)
                if 1 <= qb < QB - 1 and g == 24:
                    production(qb + 1, part=3)
                if g == 12:
                    for f in pending_bc:
                        f()
                    pending_bc.clear()
                elif g in (15, 17, 19, 21) and pending_proj:
                    pending_proj.pop(0)()
                et = expp.tile([128, HPC, 512], BF16)
                exp_bi = nc.scalar.activation(
                    out=et, in_=ks, func=AF.Exp, scale=1.0
                )
                last_exp["inst"] = exp_bi.ins

                for h in range(HPC):
                    pending_pv.append((i, qb, kb, h, et))
                while pending_pv and i - pending_pv[0][0] >= 3 + pending_pv[0][3]:
                    pv_one(*pending_pv.pop(0)[1:])
                if kb == NT - 1:
                    qsl = slice(qb * 512, (qb + 1) * 512)
                    for h in range(HPC):
                        evc, rec, bcm = make_tail(
                            po_tiles[(qb, h)], h, qsl, qb * HPC + h,
                            final=(qb == QB - 1),
                        )
                        evac_by[(qb, h)] = (evc, rec)
                        pending_bc.append(bcm)
                    for j in range(4):
                        pending_proj.append(make_proj(qb, j))
            while pending_pv:
                pv_one(*pending_pv.pop(0)[1:])
            # --- final-qb tail: keep the PE warm through the reciprocal
            # round trip (else HAM re-throttles and the projections run at
            # 1.2 GHz), and interleave per-128-block normalize with the
            # projections to shorten the serial chain ---
            for r in pending_recip:
                r()
            pending_recip.clear()
            pe_keepwarm(46)
            for j in range(4):
                for f in pending_bc:
                    f(block=j)
                pending_proj.pop(0)()
            pending_bc.clear()

    return nc


_PROGRAM = None


def _get_program():
    global _PROGRAM
    if _PROGRAM is None:
        _PROGRAM = _build_program()
    return _PROGRAM


def _bf16(a):
    import ml_dtypes

    return np.asarray(a, dtype=np.float32).astype(ml_dtypes.bfloat16)


def _prep_core_inputs(x, W_qkv, b_qkv, heads, batch):
    """Host-side slicing/relayout for one core."""
    cols = np.concatenate([np.arange(h * HD, (h + 1) * HD) for h in heads])
    # q weights/bias pre-scaled by 1/sqrt(C): scores arrive ready for exp
    w_q = W_qkv[:, cols] * SCALE          # [512, 128]
    w_k = W_qkv[:, C + cols]
    w_v = W_qkv[:, 2 * C + cols]
    # both heads packed in one M=128 projection (rows of qT = stacked heads)
    w_q = np.ascontiguousarray(w_q.reshape(NCJ, 128, HPC * HD).transpose(1, 0, 2))
    w_k = np.ascontiguousarray(w_k.reshape(NCJ, 128, HPC * HD).transpose(1, 0, 2))
    w_v = np.ascontiguousarray(
        w_v.reshape(NCJ, 128, HPC * HD).transpose(1, 0, 2))

    b_q = (b_qkv[cols] * SCALE).reshape(128, 1).astype(np.float32)
    b_k = b_qkv[C + cols].reshape(128, 1).astype(np.float32)
    xt = np.ascontiguousarray(
        x[batch].T.reshape(NCJ, 128, QB, 512).transpose(2, 1, 0, 3))
    return {
        "xt": _bf16(xt),
        "w_q": _bf16(w_q),
        "w_k": _bf16(w_k),
        "w_v": _bf16(w_v),
        "b_q": b_q,
        "b_k": b_k,
    }


def _core_w_o(W_out, heads):
    rows = np.concatenate([np.arange(h * HD, (h + 1) * HD) for h in heads])
    w = np.zeros((128, HPC, C), dtype=np.float32)
    w[0:HD] = W_out[rows].reshape(HPC, HD, C).transpose(1, 0, 2)
    return _bf16(w)


def kernel(x, W_qkv, b_qkv, W_out, b_out):
    x = np.asarray(x, dtype=np.float32)
    W_qkv = np.asarray(W_qkv, dtype=np.float32)
    b_qkv = np.asarray(b_qkv, dtype=np.float32)
    W_out = np.asarray(W_out, dtype=np.float32)
    b_out = np.asarray(b_out, dtype=np.float32)

    nc = _get_program()
    in_maps = []
    for c in range(NCORES):
        batch, hp = c // 4, c % 4
        heads = [2 * hp, 2 * hp + 1]
        im = _prep_core_inputs(x, W_qkv, b_qkv, heads, batch)
        im["w_o"] = _core_w_o(W_out, heads)
        in_maps.append(im)

    res = run_bass_kernel_spmd(nc, in_maps, core_ids=list(range(NCORES)))

    # v-bias commutes: softmax rows sum to 1, so (P @ (V + 1 b_v)) @ W_o
    # = P@V@W_o + b_v@W_o. Add b_v@W_out and b_out once on the host.
    const_row = b_qkv[2 * C:] @ W_out + b_out    # [512]
    out = np.empty((B, N, C), dtype=np.float32)
    for b in range(B):
        acc = res.results[4 * b]["out"].astype(np.float32).copy()
        for c in range(4 * b + 1, 4 * b + 4):
            acc += res.results[c]["out"]
        out[b] = acc + const_row
    return out
